# revision 12
# baseline (speedup 1.0000x reference)
"""GCNConv Trainium2 kernel (fp8 DoubleRow spmm).

Reference computation (all raw row-major reshapes):
    x_flat = x.reshape(-1, 64)                 # [960000, 64]
    h = (x_flat @ W).reshape(5000, 12288)
    agg = F @ h                                # [5000,5000] @ [5000,12288]
    out = agg.reshape(-1, 64) + bias           # [960000, 64]

Equivalently h = X2 @ blockdiag(W x192) with X2 = x.reshape(5000, 12288).

Sharding: the 12288-wide feature axis splits into 8 shards of 1536 columns,
one per NeuronCore; F and W are replicated, so there are no collectives.

The spmm dominates (614 of 622 GFLOP), so it runs in fp8e4m3 with
perf_mode=DoubleRow (K virtualized to 256, ~1.8x the bf16 MACs/cycle).
fp8's ~2% quantization noise is halved by mean-centering the filter on the
host: F = mu*ones + F', with F' in [-0.5, 0.5] quantized to fp8 and the
rank-1 term mu * colsum(h) computed exactly on the host (colsum(h) is a
cheap reduction over x) and folded into the per-column bias.

Per-core device kernel:
  phase 1: Y_c = X_c @ blockdiag(W, W) per 128-column pair in fp16 (X_c^T
           shipped pre-transposed), PSUM fp32 results stored to SBUF as
           fp8e4m3: yc [128, 40, 1536] (vertex dim on partitions).
  phase 2: agg_c = F' @ Y_c: F'^T fp8 (host-pretransposed, mean-centered,
           zero-padded to 5120, swizzled so each staging DMA is one
           contiguous 20KB read per partition) as the stationary operand;
           DoubleRow matmuls contract 256 vertices per instruction into
           512-wide fp32 PSUM chunks; bias (incl. rank-1 term) added on the
           vector engine; fp32 rows DMA'd out.
"""

import numpy as np

import concourse.bass as bass
import concourse.mybir as mybir
import concourse.tile as tile
from concourse import bacc
from concourse.bass_utils import run_bass_kernel_spmd

N_CORES = 8
NV = 5000            # vertex count
P = 128
NVP = 5120           # NV padded to a multiple of 256 (DoubleRow pairs)
KT = NVP // P        # 40 contraction / output row tiles of 128
KT2 = KT // 2        # 20 DoubleRow contraction steps of 256
COLS_TOTAL = 12288   # B*T*c_out columns of the transformed feature matrix
COLS = COLS_TOTAL // N_CORES   # 1536 per core
CIN = 64
COUT = 64
GP = COLS // P       # 12 column-pair groups (two 64-blocks each)
FREE = 512           # matmul moving free dim (one fp32 PSUM bank)
NB = COLS // FREE    # 3 chunks
M_GRP = 4            # output row tiles per F'^T staging DMA
MW = M_GRP * P       # 512
NMG = KT // M_GRP    # 10 staging groups

MM_DT = mybir.dt.float16
MM_NP = np.float16
FP8_DT = mybir.dt.float8e4
FP8_NP = mybir.dt.np(mybir.dt.float8e4)   # ml_dtypes.float8_e4m3 (max +-240)


def build_nc():
    nc = bacc.Bacc(None, target_bir_lowering=False)

    xt_d = nc.dram_tensor("xt", [COLS, NVP], MM_DT, kind="ExternalInput")
    ft = nc.dram_tensor("ft", [P, NMG, KT, MW], FP8_DT, kind="ExternalInput")
    w2 = nc.dram_tensor("w2", [P, P], MM_DT, kind="ExternalInput")
    biasb = nc.dram_tensor("biasb", [P, COLS], mybir.dt.float32, kind="ExternalInput")
    out = nc.dram_tensor("out", [NV, COLS], mybir.dt.float32, kind="ExternalOutput")

    xtr = xt_d.rearrange("(gp p) i -> gp p i", p=P)   # [12, 128, 5120]

    with tile.TileContext(nc) as tc:
        with (
            tc.tile_pool(name="const", bufs=1) as const,
            tc.tile_pool(name="ycache", bufs=1) as ypool,
            tc.tile_pool(name="fts", bufs=2) as ftsp,
            tc.tile_pool(name="xin", bufs=3) as xin,
        ):
            # DMA order matters at startup: phase 1 can begin once w2 and the
            # first X^T tile land, and the X^T prefetch chain must not sit
            # behind the bulky F'/bias transfers, so those are deferred into
            # the gp loop below.
            w2_sb = const.tile([P, P], MM_DT)
            nc.sync.dma_start(w2_sb[:], w2[:])
            xts_tiles = {}
            def stage_x(gp):
                xts = xin.tile([P, NVP], MM_DT, name="xts")
                nc.sync.dma_start(xts[:], xtr[gp])
                xts_tiles[gp] = xts
            stage_x(0)
            stage_x(1)
            stage_x(2)

            bias_sb = const.tile([P, COLS], mybir.dt.float32)
            yc = ypool.tile([P, KT, COLS], FP8_DT)

            fts_tiles = {}
            def stage_ft(mg):
                fts = ftsp.tile([P, KT, MW], FP8_DT, name="fts")
                nc.sync.dma_start(fts[:], ft[:, mg])
                fts_tiles[mg] = fts

            # ---- phase 1: yc = X @ blockdiag(W, W), X^T streamed ----
            # Four v-tiles' results share one PSUM bank so each PSUM->SBUF
            # fp8 evacuation moves [128, 512] instead of [128, 128]; the
            # per-instruction overhead of the copies otherwise throttles
            # this phase.
            with tc.tile_pool(name="ypsum", bufs=4, space="PSUM") as ypsum:
                for gp in range(GP):
                    if gp + 3 < GP:
                        stage_x(gp + 3)
                    # F'/bias prefetches ride behind the X^T chain: they are
                    # only needed once phase 2 starts.
                    if gp == 4:
                        nc.sync.dma_start(bias_sb[:], biasb[:])
                    elif gp == 6:
                        stage_ft(0)
                    elif gp == 8:
                        stage_ft(1)
                    xts = xts_tiles.pop(gp)
                    for it4 in range(KT // 4):
                        psy = ypsum.tile([P, 4 * P], mybir.dt.float32)
                        for j in range(4):
                            it = it4 * 4 + j
                            nc.tensor.matmul(
                                psy[:, j * P : (j + 1) * P],
                                xts[:, it * P : (it + 1) * P],
                                w2_sb[:],
                                start=True,
                                stop=True,
                            )
                        nc.any.tensor_copy(
                            yc[:, it4 * 4 : (it4 + 1) * 4, gp * P : (gp + 1) * P],
                            psy[:].rearrange("p (a b) -> p a b", a=4),
                        )

            # ---- phase 2: out rows = F' @ Y + bias (DoubleRow fp8) ----
            with (
                tc.tile_pool(name="osb", bufs=2) as osbp,
                tc.tile_pool(name="opsum", bufs=2, space="PSUM") as opsum,
            ):
                for mg in range(NMG):
                    if mg + 2 < NMG:
                        stage_ft(mg + 2)
                    fts = fts_tiles.pop(mg)
                    for ms in range(M_GRP):
                        m = mg * M_GRP + ms
                        psums = [
                            opsum.tile([P, FREE], mybir.dt.float32, name=f"ops{nb}")
                            for nb in range(NB)
                        ]
                        for k2 in range(KT2):
                            for nb in range(NB):
                                nc.tensor.matmul(
                                    psums[nb][:],
                                    fts[:, 2 * k2 : 2 * k2 + 2, ms * P : (ms + 1) * P],
                                    yc[:, 2 * k2 : 2 * k2 + 2, nb * FREE : (nb + 1) * FREE],
                                    start=(k2 == 0),
                                    stop=(k2 == KT2 - 1),
                                    perf_mode=mybir.MatmulPerfMode.DoubleRow,
                                )
                        osb = osbp.tile([P, COLS], mybir.dt.float32)
                        for nb in range(NB):
                            nc.vector.tensor_add(
                                osb[:, nb * FREE : (nb + 1) * FREE],
                                psums[nb][:],
                                bias_sb[:, nb * FREE : (nb + 1) * FREE],
                            )
                        rows = min(P, NV - m * P)
                        if rows > 0:
                            nc.sync.dma_start(
                                out[m * P : m * P + rows, :], osb[:rows, :]
                            )

    nc.compile()
    return nc


def prepare_in_maps(x, gcnconv_filter, weight, bias):
    x2 = np.ascontiguousarray(x, dtype=np.float32).reshape(NV, COLS_TOTAL)

    f = np.asarray(gcnconv_filter, dtype=np.float32)
    mu = float(f.mean(dtype=np.float64))
    ftp = np.zeros((NVP, NVP), dtype=np.float32)
    ftp[:NV, :NV] = (f - mu).T
    # swizzle so staging DMA mg reads [128, KT*MW] contiguously per partition:
    # ft_sw[p, mg, kt, mw] = F'^T[kt*128 + p, mg*512 + mw]
    ft_sw = np.ascontiguousarray(
        ftp.reshape(KT, P, NMG, MW).transpose(1, 2, 0, 3)
    ).astype(FP8_NP)

    w2 = np.zeros((P, P), dtype=MM_NP)
    w = np.asarray(weight, dtype=np.float32)
    w2[:CIN, :COUT] = w
    w2[CIN:, COUT:] = w

    # bias_tot[j] = bias[j % 64] + mu * colsum_h[j], with
    # colsum_h[block g] = (sum_v X2[v, g-block]) @ W  (exact, host fp64)
    colsum_x = x2.sum(axis=0, dtype=np.float64)                  # [12288]
    colsum_h = colsum_x.reshape(-1, CIN) @ w.astype(np.float64)  # [192, 64]
    bias_tot = (
        np.asarray(bias, dtype=np.float64)[None, :] + mu * colsum_h
    ).reshape(COLS_TOTAL).astype(np.float32)

    in_maps = []
    for c in range(N_CORES):
        xct = np.zeros((COLS, NVP), dtype=MM_NP)
        xct[:, :NV] = x2[:, c * COLS : (c + 1) * COLS].T
        biasb = np.ascontiguousarray(
            np.broadcast_to(
                bias_tot[None, c * COLS : (c + 1) * COLS], (P, COLS)
            ),
            dtype=np.float32,
        )
        in_maps.append({"xt": xct, "ft": ft_sw, "w2": w2, "biasb": biasb})
    return in_maps


def assemble_output(results):
    out2 = np.empty((NV, COLS_TOTAL), dtype=np.float32)
    for c in range(N_CORES):
        out2[:, c * COLS : (c + 1) * COLS] = results[c]["out"]
    return out2.reshape(NV * COLS_TOTAL // COUT, COUT)


_NC_CACHE = None


def kernel(x, gcnconv_filter, weight, bias):
    global _NC_CACHE
    if _NC_CACHE is None:
        _NC_CACHE = build_nc()
    in_maps = prepare_in_maps(x, gcnconv_filter, weight, bias)
    res = run_bass_kernel_spmd(_NC_CACHE, in_maps, core_ids=list(range(N_CORES)))
    return assemble_output(res.results)


# revision 14
# speedup vs baseline: 1.0045x; 1.0045x over previous
"""GCNConv Trainium2 kernel (fp8 DoubleRow spmm).

Reference computation (all raw row-major reshapes):
    x_flat = x.reshape(-1, 64)                 # [960000, 64]
    h = (x_flat @ W).reshape(5000, 12288)
    agg = F @ h                                # [5000,5000] @ [5000,12288]
    out = agg.reshape(-1, 64) + bias           # [960000, 64]

Equivalently h = X2 @ blockdiag(W x192) with X2 = x.reshape(5000, 12288).

Sharding: the 12288-wide feature axis splits into 8 shards of 1536 columns,
one per NeuronCore; F and W are replicated, so there are no collectives.

The spmm dominates (614 of 622 GFLOP), so it runs in fp8e4m3 with
perf_mode=DoubleRow (K virtualized to 256, ~1.8x the bf16 MACs/cycle).
fp8's ~2% quantization noise is halved by mean-centering the filter on the
host: F = mu*ones + F', with F' in [-0.5, 0.5] quantized to fp8 and the
rank-1 term mu * colsum(h) computed exactly on the host (colsum(h) is a
cheap reduction over x) and folded into the per-column bias.

Per-core device kernel:
  phase 1: Y_c = X_c @ blockdiag(W, W) per 128-column pair in fp16 (X_c^T
           shipped pre-transposed), PSUM fp32 results stored to SBUF as
           fp8e4m3: yc [128, 40, 1536] (vertex dim on partitions).
  phase 2: agg_c = F' @ Y_c: F'^T fp8 (host-pretransposed, mean-centered,
           zero-padded to 5120, swizzled so each staging DMA is one
           contiguous 20KB read per partition) as the stationary operand;
           DoubleRow matmuls contract 256 vertices per instruction into
           512-wide fp32 PSUM chunks; bias (incl. rank-1 term) added on the
           vector engine; fp32 rows DMA'd out.
"""

import numpy as np

import concourse.bass as bass
import concourse.mybir as mybir
import concourse.tile as tile
from concourse import bacc
from concourse.bass_utils import run_bass_kernel_spmd

N_CORES = 8
NV = 5000            # vertex count
P = 128
NVP = 5120           # NV padded to a multiple of 256 (DoubleRow pairs)
KT = NVP // P        # 40 contraction / output row tiles of 128
KT2 = KT // 2        # 20 DoubleRow contraction steps of 256
COLS_TOTAL = 12288   # B*T*c_out columns of the transformed feature matrix
COLS = COLS_TOTAL // N_CORES   # 1536 per core
CIN = 64
COUT = 64
GP = COLS // P       # 12 column-pair groups (two 64-blocks each)
FREE = 512           # matmul moving free dim (one fp32 PSUM bank)
NB = COLS // FREE    # 3 chunks
M_GRP = 4            # output row tiles per F'^T staging DMA
MW = M_GRP * P       # 512
NMG = KT // M_GRP    # 10 staging groups

MM_DT = mybir.dt.float16
MM_NP = np.float16
FP8_DT = mybir.dt.float8e4
FP8_NP = mybir.dt.np(mybir.dt.float8e4)   # ml_dtypes.float8_e4m3 (max +-240)


def build_nc():
    nc = bacc.Bacc(None, target_bir_lowering=False)

    xt_d = nc.dram_tensor("xt", [COLS, NVP], MM_DT, kind="ExternalInput")
    ft = nc.dram_tensor("ft", [P, NMG, KT, MW], FP8_DT, kind="ExternalInput")
    w2 = nc.dram_tensor("w2", [P, P], MM_DT, kind="ExternalInput")
    biasb = nc.dram_tensor("biasb", [P, COLS], mybir.dt.float32, kind="ExternalInput")
    out = nc.dram_tensor("out", [NV, COLS], mybir.dt.float32, kind="ExternalOutput")

    xtr = xt_d.rearrange("(gp p) i -> gp p i", p=P)   # [12, 128, 5120]

    with tile.TileContext(nc) as tc:
        with (
            tc.tile_pool(name="const", bufs=1) as const,
            tc.tile_pool(name="ycache", bufs=1) as ypool,
            tc.tile_pool(name="fts", bufs=2) as ftsp,
            tc.tile_pool(name="xin", bufs=3) as xin,
        ):
            # DMA order matters at startup: phase 1 can begin once w2 and the
            # first X^T tile land, and the X^T prefetch chain must not sit
            # behind the bulky F'/bias transfers, so those are deferred into
            # the gp loop below.
            w2_sb = const.tile([P, P], MM_DT)
            nc.sync.dma_start(w2_sb[:], w2[:])
            # gp0's X^T lands as two 2560-column chunks so its first matmuls
            # start after half the transfer; later gps stay whole-tile (their
            # DMAs hide under compute).
            X0W = NVP // 2
            x0_chunks = []
            for ci in range(2):
                t = xin.tile([P, X0W], MM_DT, name=f"x0ch{ci}")
                nc.sync.dma_start(t[:], xtr[0, :, ci * X0W : (ci + 1) * X0W])
                x0_chunks.append(t)
            xts_tiles = {}
            def stage_x(gp):
                xts = xin.tile([P, NVP], MM_DT, name="xts")
                nc.sync.dma_start(xts[:], xtr[gp])
                xts_tiles[gp] = xts
            stage_x(1)
            stage_x(2)

            bias_sb = const.tile([P, COLS], mybir.dt.float32)
            yc = ypool.tile([P, KT, COLS], FP8_DT)

            fts_tiles = {}
            def stage_ft(mg):
                fts = ftsp.tile([P, KT, MW], FP8_DT, name="fts")
                nc.sync.dma_start(fts[:], ft[:, mg])
                fts_tiles[mg] = fts

            # ---- phase 1: yc = X @ blockdiag(W, W), X^T streamed ----
            # Four v-tiles' results share one PSUM bank so each PSUM->SBUF
            # fp8 evacuation moves [128, 512] instead of [128, 128]; the
            # per-instruction overhead of the copies otherwise throttles
            # this phase.
            with tc.tile_pool(name="ypsum", bufs=4, space="PSUM") as ypsum:
                for gp in range(GP):
                    if gp + 3 < GP:
                        stage_x(gp + 3)
                    # F'/bias prefetches ride behind the X^T chain: they are
                    # only needed once phase 2 starts.
                    if gp == 4:
                        nc.sync.dma_start(bias_sb[:], biasb[:])
                    elif gp == 6:
                        stage_ft(0)
                    elif gp == 8:
                        stage_ft(1)
                    xts = xts_tiles.pop(gp) if gp > 0 else None
                    for it4 in range(KT // 4):
                        psy = ypsum.tile([P, 4 * P], mybir.dt.float32)
                        for j in range(4):
                            it = it4 * 4 + j
                            if gp == 0:
                                col = it * P
                                xsl = x0_chunks[col // X0W][
                                    :, col % X0W : col % X0W + P
                                ]
                            else:
                                xsl = xts[:, it * P : (it + 1) * P]
                            nc.tensor.matmul(
                                psy[:, j * P : (j + 1) * P],
                                xsl,
                                w2_sb[:],
                                start=True,
                                stop=True,
                            )
                        nc.any.tensor_copy(
                            yc[:, it4 * 4 : (it4 + 1) * 4, gp * P : (gp + 1) * P],
                            psy[:].rearrange("p (a b) -> p a b", a=4),
                        )

            # ---- phase 2: out rows = F' @ Y + bias (DoubleRow fp8) ----
            with (
                tc.tile_pool(name="osb", bufs=2) as osbp,
                tc.tile_pool(name="opsum", bufs=2, space="PSUM") as opsum,
            ):
                for mg in range(NMG):
                    if mg + 2 < NMG:
                        stage_ft(mg + 2)
                    fts = fts_tiles.pop(mg)
                    for ms in range(M_GRP):
                        m = mg * M_GRP + ms
                        psums = [
                            opsum.tile([P, FREE], mybir.dt.float32, name=f"ops{nb}")
                            for nb in range(NB)
                        ]
                        for k2 in range(KT2):
                            for nb in range(NB):
                                nc.tensor.matmul(
                                    psums[nb][:],
                                    fts[:, 2 * k2 : 2 * k2 + 2, ms * P : (ms + 1) * P],
                                    yc[:, 2 * k2 : 2 * k2 + 2, nb * FREE : (nb + 1) * FREE],
                                    start=(k2 == 0),
                                    stop=(k2 == KT2 - 1),
                                    perf_mode=mybir.MatmulPerfMode.DoubleRow,
                                )
                        osb = osbp.tile([P, COLS], mybir.dt.float32)
                        for nb in range(NB):
                            nc.vector.tensor_add(
                                osb[:, nb * FREE : (nb + 1) * FREE],
                                psums[nb][:],
                                bias_sb[:, nb * FREE : (nb + 1) * FREE],
                            )
                        rows = min(P, NV - m * P)
                        if rows > 0:
                            nc.sync.dma_start(
                                out[m * P : m * P + rows, :], osb[:rows, :]
                            )

    nc.compile()
    return nc


def prepare_in_maps(x, gcnconv_filter, weight, bias):
    x2 = np.ascontiguousarray(x, dtype=np.float32).reshape(NV, COLS_TOTAL)

    f = np.asarray(gcnconv_filter, dtype=np.float32)
    mu = float(f.mean(dtype=np.float64))
    ftp = np.zeros((NVP, NVP), dtype=np.float32)
    ftp[:NV, :NV] = (f - mu).T
    # swizzle so staging DMA mg reads [128, KT*MW] contiguously per partition:
    # ft_sw[p, mg, kt, mw] = F'^T[kt*128 + p, mg*512 + mw]
    ft_sw = np.ascontiguousarray(
        ftp.reshape(KT, P, NMG, MW).transpose(1, 2, 0, 3)
    ).astype(FP8_NP)

    w2 = np.zeros((P, P), dtype=MM_NP)
    w = np.asarray(weight, dtype=np.float32)
    w2[:CIN, :COUT] = w
    w2[CIN:, COUT:] = w

    # bias_tot[j] = bias[j % 64] + mu * colsum_h[j], with
    # colsum_h[block g] = (sum_v X2[v, g-block]) @ W  (exact, host fp64)
    colsum_x = x2.sum(axis=0, dtype=np.float64)                  # [12288]
    colsum_h = colsum_x.reshape(-1, CIN) @ w.astype(np.float64)  # [192, 64]
    bias_tot = (
        np.asarray(bias, dtype=np.float64)[None, :] + mu * colsum_h
    ).reshape(COLS_TOTAL).astype(np.float32)

    in_maps = []
    for c in range(N_CORES):
        xct = np.zeros((COLS, NVP), dtype=MM_NP)
        xct[:, :NV] = x2[:, c * COLS : (c + 1) * COLS].T
        biasb = np.ascontiguousarray(
            np.broadcast_to(
                bias_tot[None, c * COLS : (c + 1) * COLS], (P, COLS)
            ),
            dtype=np.float32,
        )
        in_maps.append({"xt": xct, "ft": ft_sw, "w2": w2, "biasb": biasb})
    return in_maps


def assemble_output(results):
    out2 = np.empty((NV, COLS_TOTAL), dtype=np.float32)
    for c in range(N_CORES):
        out2[:, c * COLS : (c + 1) * COLS] = results[c]["out"]
    return out2.reshape(NV * COLS_TOTAL // COUT, COUT)


_NC_CACHE = None


def kernel(x, gcnconv_filter, weight, bias):
    global _NC_CACHE
    if _NC_CACHE is None:
        _NC_CACHE = build_nc()
    in_maps = prepare_in_maps(x, gcnconv_filter, weight, bias)
    res = run_bass_kernel_spmd(_NC_CACHE, in_maps, core_ids=list(range(N_CORES)))
    return assemble_output(res.results)


# revision 16
# speedup vs baseline: 1.0102x; 1.0057x over previous
"""GCNConv Trainium2 kernel (fp8 DoubleRow spmm).

Reference computation (all raw row-major reshapes):
    x_flat = x.reshape(-1, 64)                 # [960000, 64]
    h = (x_flat @ W).reshape(5000, 12288)
    agg = F @ h                                # [5000,5000] @ [5000,12288]
    out = agg.reshape(-1, 64) + bias           # [960000, 64]

Equivalently h = X2 @ blockdiag(W x192) with X2 = x.reshape(5000, 12288).

Sharding: the 12288-wide feature axis splits into 8 shards of 1536 columns,
one per NeuronCore; F and W are replicated, so there are no collectives.

The spmm dominates (614 of 622 GFLOP), so it runs in fp8e4m3 with
perf_mode=DoubleRow (K virtualized to 256, ~1.8x the bf16 MACs/cycle).
fp8's ~2% quantization noise is halved by mean-centering the filter on the
host: F = mu*ones + F', with F' in [-0.5, 0.5] quantized to fp8 and the
rank-1 term mu * colsum(h) computed exactly on the host (colsum(h) is a
cheap reduction over x) and folded into the per-column bias.

Per-core device kernel:
  phase 1: Y_c = X_c @ blockdiag(W, W) per 128-column pair in fp16 (X_c^T
           shipped pre-transposed), PSUM fp32 results stored to SBUF as
           fp8e4m3: yc [128, 40, 1536] (vertex dim on partitions).
  phase 2: agg_c = F' @ Y_c: F'^T fp8 (host-pretransposed, mean-centered,
           zero-padded to 5120, swizzled so each staging DMA is one
           contiguous 20KB read per partition) as the stationary operand;
           DoubleRow matmuls contract 256 vertices per instruction into
           512-wide fp32 PSUM chunks; bias (incl. rank-1 term) added on the
           vector engine; fp32 rows DMA'd out.
"""

import numpy as np

import concourse.bass as bass
import concourse.mybir as mybir
import concourse.tile as tile
from concourse import bacc
from concourse.bass_utils import run_bass_kernel_spmd

N_CORES = 8
NV = 5000            # vertex count
P = 128
NVP = 5120           # NV padded to a multiple of 256 (DoubleRow pairs)
KT = NVP // P        # 40 contraction / output row tiles of 128
KT2 = KT // 2        # 20 DoubleRow contraction steps of 256
COLS_TOTAL = 12288   # B*T*c_out columns of the transformed feature matrix
COLS = COLS_TOTAL // N_CORES   # 1536 per core
CIN = 64
COUT = 64
GP = COLS // P       # 12 column-pair groups (two 64-blocks each)
FREE = 512           # matmul moving free dim (one fp32 PSUM bank)
NB = COLS // FREE    # 3 chunks
M_GRP = 4            # output row tiles per F'^T staging DMA
MW = M_GRP * P       # 512
NMG = KT // M_GRP    # 10 staging groups

MM_DT = mybir.dt.float16
MM_NP = np.float16
FP8_DT = mybir.dt.float8e4
FP8_NP = mybir.dt.np(mybir.dt.float8e4)   # ml_dtypes.float8_e4m3 (max +-240)


def build_nc():
    nc = bacc.Bacc(None, target_bir_lowering=False)

    xt_d = nc.dram_tensor("xt", [COLS, NVP], MM_DT, kind="ExternalInput")
    ft = nc.dram_tensor("ft", [P, NMG, KT, MW], FP8_DT, kind="ExternalInput")
    w2 = nc.dram_tensor("w2", [P, P], MM_DT, kind="ExternalInput")
    biasb = nc.dram_tensor("biasb", [P, COLS], mybir.dt.float32, kind="ExternalInput")
    out = nc.dram_tensor("out", [NV, COLS], mybir.dt.float32, kind="ExternalOutput")

    xtr = xt_d.rearrange("(gp p) i -> gp p i", p=P)   # [12, 128, 5120]

    with tile.TileContext(nc) as tc:
        with (
            tc.tile_pool(name="const", bufs=1) as const,
            tc.tile_pool(name="ycache", bufs=1) as ypool,
            tc.tile_pool(name="fts", bufs=2) as ftsp,
            tc.tile_pool(name="xin", bufs=3) as xin,
        ):
            # DMA order matters at startup: phase 1 can begin once w2 and the
            # first X^T tile land, and the X^T prefetch chain must not sit
            # behind the bulky F'/bias transfers, so those are deferred into
            # the gp loop below.
            w2_sb = const.tile([P, P], MM_DT)
            nc.sync.dma_start(w2_sb[:], w2[:])
            xts_tiles = {}
            def stage_x(gp):
                xts = xin.tile([P, NVP], MM_DT, name="xts")
                nc.sync.dma_start(xts[:], xtr[gp])
                xts_tiles[gp] = xts
            stage_x(0)
            stage_x(1)
            stage_x(2)

            bias_sb = const.tile([P, COLS], mybir.dt.float32)
            yc = ypool.tile([P, KT, COLS], FP8_DT)

            fts_tiles = {}
            def stage_ft(mg):
                fts = ftsp.tile([P, KT, MW], FP8_DT, name="fts")
                nc.sync.dma_start(fts[:], ft[:, mg])
                fts_tiles[mg] = fts

            # ---- phase 1: yc = X @ blockdiag(W, W), X^T streamed ----
            # Four v-tiles' results share one PSUM bank so each PSUM->SBUF
            # fp8 evacuation moves [128, 512] instead of [128, 128]; the
            # per-instruction overhead of the copies otherwise throttles
            # this phase.
            with tc.tile_pool(name="ypsum", bufs=4, space="PSUM") as ypsum:
                for gp in range(GP):
                    if gp + 3 < GP:
                        stage_x(gp + 3)
                    # F'/bias prefetches ride behind the X^T chain: they are
                    # only needed once phase 2 starts.
                    if gp == 4:
                        nc.sync.dma_start(bias_sb[:], biasb[:])
                    elif gp == 6:
                        stage_ft(0)
                    elif gp == 8:
                        stage_ft(1)
                    xts = xts_tiles.pop(gp)
                    for it4 in range(KT // 4):
                        psy = ypsum.tile([P, 4 * P], mybir.dt.float32)
                        for j in range(4):
                            it = it4 * 4 + j
                            nc.tensor.matmul(
                                psy[:, j * P : (j + 1) * P],
                                xts[:, it * P : (it + 1) * P],
                                w2_sb[:],
                                start=True,
                                stop=True,
                            )
                        nc.any.tensor_copy(
                            yc[:, it4 * 4 : (it4 + 1) * 4, gp * P : (gp + 1) * P],
                            psy[:].rearrange("p (a b) -> p a b", a=4),
                        )

            # ---- phase 2: out rows = F' @ Y + bias (DoubleRow fp8) ----
            with (
                tc.tile_pool(name="osb", bufs=2) as osbp,
                tc.tile_pool(name="opsum", bufs=2, space="PSUM") as opsum,
            ):
                for mg in range(NMG):
                    if mg + 2 < NMG:
                        stage_ft(mg + 2)
                    fts = fts_tiles.pop(mg)
                    for ms in range(M_GRP):
                        m = mg * M_GRP + ms
                        psums = [
                            opsum.tile([P, FREE], mybir.dt.float32, name=f"ops{nb}")
                            for nb in range(NB)
                        ]
                        for k2 in range(KT2):
                            for nb in range(NB):
                                nc.tensor.matmul(
                                    psums[nb][:],
                                    fts[:, 2 * k2 : 2 * k2 + 2, ms * P : (ms + 1) * P],
                                    yc[:, 2 * k2 : 2 * k2 + 2, nb * FREE : (nb + 1) * FREE],
                                    start=(k2 == 0),
                                    stop=(k2 == KT2 - 1),
                                    perf_mode=mybir.MatmulPerfMode.DoubleRow,
                                )
                        osb = osbp.tile([P, COLS], mybir.dt.float32)
                        for nb in range(NB):
                            nc.vector.tensor_add(
                                osb[:, nb * FREE : (nb + 1) * FREE],
                                psums[nb][:],
                                bias_sb[:, nb * FREE : (nb + 1) * FREE],
                            )
                        rows = min(P, NV - m * P)
                        if rows > 0:
                            nc.sync.dma_start(
                                out[m * P : m * P + rows, :], osb[:rows, :]
                            )

    nc.compile()
    return nc


def prepare_in_maps(x, gcnconv_filter, weight, bias):
    x2 = np.ascontiguousarray(x, dtype=np.float32).reshape(NV, COLS_TOTAL)

    f = np.asarray(gcnconv_filter, dtype=np.float32)
    mu = float(f.mean(dtype=np.float64))
    ftp = np.zeros((NVP, NVP), dtype=np.float32)
    ftp[:NV, :NV] = (f - mu).T
    # swizzle so staging DMA mg reads [128, KT*MW] contiguously per partition:
    # ft_sw[p, mg, kt, mw] = F'^T[kt*128 + p, mg*512 + mw]
    ft_sw = np.ascontiguousarray(
        ftp.reshape(KT, P, NMG, MW).transpose(1, 2, 0, 3)
    ).astype(FP8_NP)

    w2 = np.zeros((P, P), dtype=MM_NP)
    w = np.asarray(weight, dtype=np.float32)
    w2[:CIN, :COUT] = w
    w2[CIN:, COUT:] = w

    # bias_tot[j] = bias[j % 64] + mu * colsum_h[j], with
    # colsum_h[block g] = (sum_v X2[v, g-block]) @ W  (exact, host fp64)
    colsum_x = x2.sum(axis=0, dtype=np.float64)                  # [12288]
    colsum_h = colsum_x.reshape(-1, CIN) @ w.astype(np.float64)  # [192, 64]
    bias_tot = (
        np.asarray(bias, dtype=np.float64)[None, :] + mu * colsum_h
    ).reshape(COLS_TOTAL).astype(np.float32)

    in_maps = []
    for c in range(N_CORES):
        xct = np.zeros((COLS, NVP), dtype=MM_NP)
        xct[:, :NV] = x2[:, c * COLS : (c + 1) * COLS].T
        biasb = np.ascontiguousarray(
            np.broadcast_to(
                bias_tot[None, c * COLS : (c + 1) * COLS], (P, COLS)
            ),
            dtype=np.float32,
        )
        in_maps.append({"xt": xct, "ft": ft_sw, "w2": w2, "biasb": biasb})
    return in_maps


def assemble_output(results):
    out2 = np.empty((NV, COLS_TOTAL), dtype=np.float32)
    for c in range(N_CORES):
        out2[:, c * COLS : (c + 1) * COLS] = results[c]["out"]
    return out2.reshape(NV * COLS_TOTAL // COUT, COUT)


_NC_CACHE = None


def kernel(x, gcnconv_filter, weight, bias):
    global _NC_CACHE
    if _NC_CACHE is None:
        _NC_CACHE = build_nc()
    in_maps = prepare_in_maps(x, gcnconv_filter, weight, bias)
    res = run_bass_kernel_spmd(_NC_CACHE, in_maps, core_ids=list(range(N_CORES)))
    return assemble_output(res.results)
